# revision 18
# baseline (speedup 1.0000x reference)
"""Multi-head attention kernel for Trainium2, batch-parallel across 8 NeuronCores.

Reference (per batch element b, one core each):
  qk = x @ W_qk.T ; q,k = split(qk) ; v = x @ W_v.T
  q,k,v -> [h, n, d] ; q += pos_h ; k += pos_h
  S = q @ k.T * DIM**-0.5 ; mask = outer(m, m) ; masked -> -inf
  P = softmax(S) ; O = P @ v ; out = merge_heads(O) @ W_out.T + b_out

Device strategy (per core), v4:
  - Phase 1 (prep): x^T kept in bf16 (for V=x@W_v.T) and fp8e4 (for the q/k
    projection, fp8 DoubleRow with W_qk prescaled by 32 -- 256-deep
    contraction per pass).  Per-pair pos/W_qk loads are prefetched two pairs
    ahead so the single Sync DMA queue never head-of-line-blocks the PE;
    transpose evictions are split between ACT and DVE.
  - Phase 2 (attention): per head pair, software-pipelined over 8 j-tiles:
    scores on PE (kT-A rows 0-63 / kT-B rows 64-127 alternate so LDWEIGHTS
    hides), exp on ACT (mask bias folded in, scale/1024), PV accumulating
    [65,1024] in PSUM (ones col = row sums).  Normalization is DMA-free:
    reciprocal+mask on the [1,N] sums row, gpsimd broadcast, one mul + one
    scalar_tensor_tensor per head.
  - Phase 3: out projection; four row-tiles accumulate pairs 0-6 early
    (overlapping the last pair's normalization) so PE never idles/cools.
"""
import os
import sys

sys.path.insert(0, "/opt/trn_rl_repo")

import numpy as np
from contextlib import ExitStack

DBG = os.environ.get("KDBG", "")

B, N, DIM, H = 8, 1024, 1024, 16
D = DIM // H          # 64
E = D + 1             # V_aug block (64 cols of V + ones column)
P = 128
NT = N // P           # 8 n-tiles
KT = DIM // P         # 8 k-tiles
NPAIR = H // 2        # 8 head pairs
SCALE = DIM ** (-0.5)
MB = 30.0             # mask bias magnitude: bias_j = 30*m - 30 in {0, -30}
MMN = 512             # moving free-dim per matmul (single-bank PSUM writes)
WS = 32.0             # fp8 prescale on W_qk / pos (q' = 32 q); exp scale /1024

_NC = None


def _build():
    import concourse.bacc as bacc
    import concourse.bass as bass
    import concourse.mybir as mybir
    import concourse.tile as tile
    from concourse.masks import make_identity

    f32 = mybir.dt.float32
    bf16 = mybir.dt.bfloat16
    f8 = mybir.dt.float8e4
    AF = mybir.ActivationFunctionType
    OP = mybir.AluOpType
    DRW = mybir.MatmulPerfMode.DoubleRow
    ts = bass.ts

    nc = bacc.Bacc()
    x_d = nc.declare_dram_parameter("x", [N, DIM], f32, isOutput=False)
    pos_d = nc.declare_dram_parameter("pos", [N, DIM], f32, isOutput=False)
    maskf_d = nc.declare_dram_parameter("maskf", [N], f32, isOutput=False)
    wqk_d = nc.declare_dram_parameter("W_qk", [2 * DIM, DIM], f32, isOutput=False)
    wv_d = nc.declare_dram_parameter("W_v", [DIM, DIM], f32, isOutput=False)
    wout_d = nc.declare_dram_parameter("W_out", [DIM, DIM], f32, isOutput=False)
    b_d = nc.declare_dram_parameter("b_out", [DIM], f32, isOutput=False)
    out_d = nc.declare_dram_parameter("out", [N, DIM], f32, isOutput=True)

    with ExitStack() as ctx:
        tc = ctx.enter_context(tile.TileContext(nc))
        sing = ctx.enter_context(tc.tile_pool(name="sing", bufs=1))
        expp = ctx.enter_context(tc.tile_pool(name="expp", bufs=4))
        oasp = ctx.enter_context(tc.tile_pool(name="oasp", bufs=2))
        ps_big = ctx.enter_context(tc.tile_pool(name="ps_big", bufs=2, space="PSUM"))

        def mm(out_ap, lhsT, rhs, start, stop):
            """matmul with the free dim chunked to MMN columns (rhs 2D [K, n])."""
            n = rhs.shape[-1]
            if n <= MMN:
                nc.tensor.matmul(out_ap, lhsT, rhs, start=start, stop=stop)
            else:
                for c0 in range(0, n, MMN):
                    nc.tensor.matmul(out_ap[:, c0:c0 + MMN], lhsT,
                                     rhs[:, c0:c0 + MMN], start=start, stop=stop)

        # ---------- constants / small prep ----------
        mstage = sing.tile([P, NT], f32, tag="mstage")
        nc.sync.dma_start(out=mstage, in_=maskf_d.rearrange("(c p) -> p c", c=NT))
        bias_j = sing.tile([P, NT], f32, tag="bias_j")
        nc.scalar.activation(bias_j, mstage, AF.Copy, bias=-MB, scale=MB)
        # preload the exp table set off the critical path
        dume = sing.tile([1, 8], f32, tag="dume")
        nc.scalar.activation(dume, mstage[0:1, :], AF.Exp)
        m_row = sing.tile([1, N], f32, tag="m_row")
        nc.sync.dma_start(out=m_row, in_=maskf_d[:])
        omm_row = sing.tile([1, N], f32, tag="omm_row")
        nc.vector.tensor_scalar(omm_row, m_row, -1.0, 1.0, OP.mult, OP.add)
        ommb = sing.tile([D, N], f32, tag="ommb")
        nc.gpsimd.partition_broadcast(ommb, omm_row)
        b_row = sing.tile([1, DIM], f32, tag="b_row")
        nc.sync.dma_start(out=b_row, in_=b_d[:])
        b_bcast = sing.tile([P, DIM], f32, tag="b_bcast")
        nc.gpsimd.partition_broadcast(b_bcast, b_row)

        # ---------- persistent data tiles ----------
        V_sb = [sing.tile([P, H * E], bf16, tag=f"V{nt}", name=f"V{nt}")
                for nt in range(NT)]
        qT_all = [sing.tile([P, N], bf16, tag=f"qT{t}", name=f"qT{t}")
                  for t in range(NPAIR)]
        kT_all = [sing.tile([P, N], bf16, tag=f"kT{t}", name=f"kT{t}")
                  for t in range(NPAIR)]
        woutT = sing.tile([P, KT, DIM], bf16, tag="woutT")   # [c, kc, od]
        otfull = [sing.tile([P, N], bf16, tag=f"otf{t}", name=f"otf{t}")
                  for t in range(NPAIR)]
        mean_cols = sing.tile([D, H], f32, tag="mean_cols")

        with (tc.tile_pool(name="p1", bufs=1) as p1,
              tc.tile_pool(name="rowload", bufs=6) as rowload,
              tc.tile_pool(name="woldp", bufs=8) as woldp,
              tc.tile_pool(name="posld", bufs=3) as posld,
              tc.tile_pool(name="wstage", bufs=2) as wstage,
              tc.tile_pool(name="posp", bufs=2) as posp,
              tc.tile_pool(name="ps_tp", bufs=2, space="PSUM") as ps_tp,
              tc.tile_pool(name="ps_sm", bufs=2, space="PSUM") as ps_sm):
            ident_bf = sing.tile([P, P], bf16, tag="ident_bf")
            make_identity(nc, ident_bf)

            # x^T fp8 persists through the DR projections
            xT_f8 = p1.tile([P, KT, N], f8, tag="xT_f8")

            def load_cast(dram_ap, scale=None, pool=None, ldb_bufs=4,
                          eng=None):
                pool = pool or rowload
                r = rowload.tile([P, DIM], f32, tag="ld", name="ld", bufs=2)
                nc.sync.dma_start(out=r, in_=dram_ap)
                bt = pool.tile([P, DIM], bf16, tag="ldb", name="ldb",
                               bufs=ldb_bufs)
                if eng == "gpsimd":
                    nc.gpsimd.tensor_copy(bt, r)
                elif scale is None:
                    nc.scalar.activation(bt, r, AF.Copy)
                else:
                    nc.scalar.activation(bt, r, AF.Copy, scale=scale)
                return bt

            def tp4(src_bf, kc0, dve_dsts, act_dsts=()):
                """Transpose 4 [P,P] blocks (cols kc0*P..) into one PSUM bank,
                then evict once per dst AP ([P, 4, P] view)."""
                tp = ps_tp.tile([P, 4 * P], bf16, tag="tp", name="tp")
                for i in range(4):
                    nc.tensor.matmul(tp[:, ts(i, P)], src_bf[:, ts(kc0 + i, P)],
                                     ident_bf, is_transpose=True,
                                     start=(i == 0), stop=(i == 3),
                                     skip_group_check=True)
                tpv = tp.rearrange("p (k c) -> p k c", c=P)
                for dst in dve_dsts:
                    nc.vector.tensor_copy(dst, tpv)
                for dst in act_dsts:
                    nc.scalar.activation(dst, tpv, AF.Copy)
            const1 = sing.tile([P, H], f32, tag="const1")
            nc.vector.memset(const1, 1.0)
            for nt in range(NT):
                ones_ap = V_sb[nt].rearrange("p (h e) -> p h e", e=E)[:, :, D:E]
                nc.vector.tensor_copy(ones_ap.squeeze(), const1)

            with tc.tile_pool(name="vps", bufs=1) as vps:
                # x^T bf16 and W_v^T live only until the V projection
                xT_bf = vps.tile([P, KT, N], bf16, tag="xT_bf")   # [c, kc, i]
                wvT = vps.tile([P, KT, DIM], bf16, tag="wvT")     # [c, kc, vd]
                for nt in range(NT):
                    xb = load_cast(x_d[ts(nt, P), :])
                    for g in range(2):
                        tp4(xb, 4 * g,
                            [xT_bf[:, 4 * g:4 * g + 4, ts(nt, P)]],
                            [xT_f8[:, 4 * g:4 * g + 4, ts(nt, P)]])
                    wb = load_cast(wv_d[ts(nt, P), :])
                    for g in range(2):
                        tp4(wb, 4 * g, [wvT[:, 4 * g:4 * g + 4, ts(nt, P)]])

                # ---------- V = x @ W_v.T (stored [V_h | 1] per head) ----------
                for nt in range(NT):
                    acc = ps_big.tile([P, N], f32, tag="st", name="vacc")
                    for kc in range(KT):
                        mm(acc, xT_bf[:, kc, ts(nt, P)], wvT[:, kc, :],
                           start=(kc == 0), stop=(kc == KT - 1))
                    dst = V_sb[nt].rearrange("p (h e) -> p h e", e=E)[:, :, 0:D]
                    nc.vector.tensor_copy(dst,
                                          acc.rearrange("p (h d) -> p h d", d=D))

            # ---------- per-pair load prefetch helpers ----------
            def pair_loads(t):
                pr = posld.tile([P, N], f32, tag="posr", name="posr", bufs=2)
                nc.sync.dma_start(
                    out=pr,
                    in_=pos_d.rearrange("(nt p) c -> p nt c", nt=NT)[:, :, ts(t, P)])
                pb = posld.tile([P, N], bf16, tag="posb", name="posb", bufs=2)
                nc.scalar.activation(pb, pr, AF.Copy, scale=WS)
                wq = load_cast(wqk_d[t * P:(t + 1) * P, :], scale=WS)
                wk = load_cast(wqk_d[DIM + t * P:DIM + (t + 1) * P, :], scale=WS)
                return pb, wq, wk

            loads = {0: pair_loads(0), 1: pair_loads(1)}

            # ---------- mean over sequence of V_aug ----------
            ones_col = sing.tile([P, 1], bf16, tag="ones_col")
            constN = sing.tile([P, 1], f32, tag="constN")
            nc.vector.memset(constN, 1.0 / N)
            nc.vector.tensor_copy(ones_col, constN)
            mean_sb = sing.tile([1, H * E], f32, tag="mean_sb")
            for c0, cs in ((0, 512), (512, 512), (1024, H * E - 1024)):
                mp = ps_sm.tile([P, 512], f32, tag="mp", name="mp")
                for nt in range(NT):
                    nc.tensor.matmul(mp[0:1, 0:cs], ones_col, V_sb[nt][:, c0:c0 + cs],
                                     start=(nt == 0), stop=(nt == NT - 1))
                nc.vector.tensor_copy(mean_sb[:, c0:c0 + cs], mp[0:1, 0:cs])
            for h in range(H):
                nc.sync.dma_start(out=mean_cols[:, h:h + 1],
                                  in_=mean_sb[0:1, h * E:h * E + D])

            # ---------- q^T / k^T: fp8 DoubleRow projection + pos add ----------
            wout_b = []
            for t in range(NPAIR):
                if t + 2 < NPAIR:
                    loads[t + 2] = pair_loads(t + 2)
                if t == 4:   # W_out rows: DMA late, cast on idle gpsimd
                    for rt in range(NT):
                        wout_b.append(load_cast(wout_d[ts(rt, P), :], pool=woldp,
                                                ldb_bufs=8, eng="gpsimd"))
                pb, wq, wk = loads.pop(t)
                posT = posp.tile([P, N], bf16, tag="posT", name="posT")
                for g in range(2):
                    tp4(pb, 4 * g,
                        [posT.rearrange("p (nt c) -> p nt c", c=P)[:, 4 * g:4 * g + 4, :]])
                for wqb, wt in ((wq, qT_all[t]), (wk, kT_all[t])):
                    w8 = wstage.tile([P, KT, P], f8, tag="w8", name="w8")
                    tp4(wqb, 0, [w8[:, 0:4, :]])
                    tp4(wqb, 4, [], [w8[:, 4:8, :]])
                    acc = ps_big.tile([P, N], f32, tag="st", name="qacc")
                    for c in range(KT // 2):
                        lhs = w8[:, 2 * c:2 * c + 2, :]
                        for ch in range(2):
                            nc.tensor.matmul(
                                acc[:, ts(ch, MMN)], lhs,
                                xT_f8[:, 2 * c:2 * c + 2, ts(ch, MMN)],
                                start=(c == 0), stop=(c == KT // 2 - 1),
                                perf_mode=DRW)
                    nc.vector.tensor_add(wt, acc, posT)

            # ---------- W_out^T (staged for phase 3) ----------
            for rt in range(NT):
                for g in range(2):
                    tp4(wout_b[rt], 4 * g, [woutT[:, 4 * g:4 * g + 4, ts(rt, P)]])

            if DBG == "qT":
                dbg1 = sing.tile([P, N], f32, tag="dbg1")
                nc.vector.tensor_copy(dbg1, qT_all[0])
                nc.sync.dma_start(out=out_d[0:P, :], in_=dbg1)
                nc.vector.tensor_copy(dbg1, kT_all[0])
                nc.sync.dma_start(out=out_d[P:2 * P, :], in_=dbg1)
            if DBG == "V":
                dbg1 = sing.tile([P, N], f32, tag="dbg1")
                nc.vector.tensor_copy(dbg1, V_sb[0][:, 0:N])
                nc.sync.dma_start(out=out_d[0:P, :], in_=dbg1)

        # ---------- attention ----------
        EXPS = SCALE / (WS * WS)

        with (tc.tile_pool(name="normp", bufs=2) as normp,
              tc.tile_pool(name="ps_oa", bufs=2, space="PSUM") as ps_oa):

            def norm_head(t, hs, oa):
                """Normalize head (2t+hs) from PSUM oa [65, N] into otfull[t]."""
                h = 2 * t + hs
                oaS = oasp.tile([E, N], bf16, tag="oaS", name="oaS")
                nc.vector.tensor_copy(oaS, oa)  # frees the PSUM banks
                r_row = normp.tile([1, N], f32, tag="r_row")
                nc.vector.reciprocal(r_row, oaS[D:D + 1, :])
                nc.vector.tensor_mul(r_row, r_row, m_row)
                rmb = normp.tile([D, N], f32, tag="rmb")
                nc.gpsimd.partition_broadcast(rmb, r_row)
                t1 = normp.tile([D, N], f32, tag="t1")
                nc.vector.tensor_mul(t1, oaS[0:D, :], rmb)
                if hs == 0:
                    nc.vector.scalar_tensor_tensor(
                        otfull[t][0:D, :], ommb, mean_cols[:, h:h + 1], t1,
                        OP.mult, OP.add)
                else:
                    hscr = normp.tile([D, N], bf16, tag="hscr")
                    nc.vector.scalar_tensor_tensor(
                        hscr, ommb, mean_cols[:, h:h + 1], t1, OP.mult, OP.add)
                    nc.sync.dma_start(out=otfull[t][D:2 * D, :], in_=hscr)

            def pv(t, prev, stop):
                pj, pexA, pexB = prev
                hA, hB = 2 * t, 2 * t + 1
                mm(oa_ab[0], V_sb[pj][:, hA * E:(hA + 1) * E], pexA,
                   start=(pj == 0), stop=stop)
                mm(oa_ab[1], V_sb[pj][:, hB * E:(hB + 1) * E], pexB,
                   start=(pj == 0), stop=stop)

            def oproj_mm(acc, nt, kcs, start):
                for kc in kcs:
                    mm(acc, otfull[kc][:, ts(nt, P)], woutT[:, kc, :],
                       start=(start and kc == kcs[0]), stop=(kc == KT - 1))

            def oproj_fin(acc, nt):
                ostage = normp.tile([P, N], f32, tag="ostage", name="ostage")
                nc.vector.tensor_add(ostage, acc, b_bcast)
                nc.sync.dma_start(out=out_d[ts(nt, P), :], in_=ostage)

            oacc4 = []
            for t in range(NPAIR):
                qT, kT = qT_all[t], kT_all[t]
                oa_ab = (ps_oa.tile([E, N], f32, tag="oa", name="oaA"),
                         ps_oa.tile([E, N], f32, tag="oa", name="oaB"))
                prev = None
                for jt in range(NT):
                    stA = ps_big.tile([P, N], f32, tag="st", name="stA")
                    mm(stA, kT[0:D, ts(jt, P)], qT[0:D, :], start=True, stop=True)
                    stB = ps_big.tile([P, N], f32, tag="st", name="stB")
                    mm(stB, kT[D:P, ts(jt, P)], qT[D:P, :], start=True, stop=True)
                    if prev is not None:
                        pv(t, prev, stop=False)
                    exA = expp.tile([P, N], bf16, tag="ex", name="exA")
                    nc.scalar.activation(exA, stA, AF.Exp,
                                         bias=bias_j[:, jt:jt + 1], scale=EXPS)
                    exB = expp.tile([P, N], bf16, tag="ex", name="exB")
                    nc.scalar.activation(exB, stB, AF.Exp,
                                         bias=bias_j[:, jt:jt + 1], scale=EXPS)
                    if DBG == "ex" and t == 0 and jt == 0:
                        dbg1 = sing.tile([P, N], f32, tag="dbg1")
                        nc.vector.tensor_copy(dbg1, stA)
                        nc.sync.dma_start(out=out_d[0:P, :], in_=dbg1)
                        dbg2 = sing.tile([P, N], f32, tag="dbg2")
                        nc.vector.tensor_copy(dbg2, exA)
                        nc.sync.dma_start(out=out_d[P:2 * P, :], in_=dbg2)
                    prev = (jt, exA, exB)
                pv(t, prev, stop=True)
                last = t == NPAIR - 1 and not DBG
                if last:
                    # keep PE busy during the last pair's normalization:
                    # accumulate pairs 0..6 of the first four out row-tiles
                    for nt in range(2):
                        acc = ps_big.tile([P, N], f32, tag="st", name="oacc")
                        oproj_mm(acc, nt, list(range(KT - 1)), start=True)
                        oacc4.append(acc)
                # for the last pair normalize head B first: its otfull write
                # goes through a DMA and gates the final out-proj kc
                for hs in ((1, 0) if last else (0, 1)):
                    norm_head(t, hs, oa_ab[hs])
                if last:
                    # reuse the oa slots (freed by the oaS evictions just above)
                    for nt in range(2, 4):
                        acc = ps_oa.tile([P, N], f32, tag="oa", name="oacc2")
                        oproj_mm(acc, nt, list(range(KT - 1)), start=True)
                        oacc4.append(acc)

            if DBG == "ot":
                dbg3 = sing.tile([P, N], f32, tag="dbg3")
                for kc in range(KT):
                    nc.vector.tensor_copy(dbg3, otfull[kc])
                    nc.sync.dma_start(out=out_d[ts(kc, P), :], in_=dbg3)

            # ---------- out projection (remaining) ----------
            if not DBG:
                for nt in range(4):
                    oproj_mm(oacc4[nt], nt, [KT - 1], start=False)
                    oproj_fin(oacc4[nt], nt)
                for nt in range(4, NT):
                    acc = ps_big.tile([P, N], f32, tag="st", name="oacc")
                    oproj_mm(acc, nt, list(range(KT)), start=True)
                    oproj_fin(acc, nt)

    nc.finalize()
    return nc


def kernel(x, mask, pos, W_qk, W_v, W_out, b_out):
    global _NC
    from concourse.bass_utils import run_bass_kernel_spmd

    if _NC is None:
        _NC = _build()

    x = np.ascontiguousarray(x, dtype=np.float32)
    pos = np.ascontiguousarray(pos, dtype=np.float32)
    maskf = np.concatenate(
        [np.ones((B, 1), np.float32), np.asarray(mask).astype(np.float32)], axis=1)
    W_qk = np.ascontiguousarray(W_qk, dtype=np.float32)
    W_v = np.ascontiguousarray(W_v, dtype=np.float32)
    W_out = np.ascontiguousarray(W_out, dtype=np.float32)
    b_out = np.ascontiguousarray(b_out, dtype=np.float32)

    in_maps = [
        {"x": x[b], "pos": pos[b], "maskf": maskf[b], "W_qk": W_qk,
         "W_v": W_v, "W_out": W_out, "b_out": b_out}
        for b in range(B)
    ]
    res = run_bass_kernel_spmd(_NC, in_maps, core_ids=list(range(B)))
    return np.stack([res.results[b]["out"] for b in range(B)]).astype(np.float32)


# revision 23
# speedup vs baseline: 1.1900x; 1.1900x over previous
"""Multi-head attention kernel for Trainium2, batch-parallel across 8 NeuronCores.

Reference (per batch element b, one core each):
  qk = x @ W_qk.T ; q,k = split(qk) ; v = x @ W_v.T
  q,k,v -> [h, n, d] ; q += pos_h ; k += pos_h
  S = q @ k.T * DIM**-0.5 ; mask = outer(m, m) ; masked -> -inf
  P = softmax(S) ; O = P @ v ; out = merge_heads(O) @ W_out.T + b_out

Device strategy (per core), v4:
  - Phase 1 (prep): x^T kept in bf16 (for V=x@W_v.T) and fp8e4 (for the q/k
    projection, fp8 DoubleRow with W_qk prescaled by 32 -- 256-deep
    contraction per pass).  Per-pair pos/W_qk loads are prefetched two pairs
    ahead so the single Sync DMA queue never head-of-line-blocks the PE;
    transpose evictions are split between ACT and DVE.
  - Phase 2 (attention): per head pair, software-pipelined over 8 j-tiles:
    scores on PE (kT-A rows 0-63 / kT-B rows 64-127 alternate so LDWEIGHTS
    hides), exp on ACT (mask bias folded in, scale/1024), PV accumulating
    [65,1024] in PSUM (ones col = row sums).  Normalization is DMA-free:
    reciprocal+mask on the [1,N] sums row, gpsimd broadcast, one mul + one
    scalar_tensor_tensor per head.
  - Phase 3: out projection; four row-tiles accumulate pairs 0-6 early
    (overlapping the last pair's normalization) so PE never idles/cools.
"""
import os
import sys

sys.path.insert(0, "/opt/trn_rl_repo")

import numpy as np
from contextlib import ExitStack

DBG = os.environ.get("KDBG", "")

B, N, DIM, H = 8, 1024, 1024, 16
D = DIM // H          # 64
E = D + 1             # V_aug block (64 cols of V + ones column)
P = 128
NT = N // P           # 8 n-tiles
KT = DIM // P         # 8 k-tiles
NPAIR = H // 2        # 8 head pairs
SCALE = DIM ** (-0.5)
MB = 30.0             # mask bias magnitude: bias_j = 30*m - 30 in {0, -30}
MMN = 512             # moving free-dim per matmul (single-bank PSUM writes)
WS = 32.0             # fp8 prescale on W_qk / pos (q' = 32 q); exp scale /1024

_NC = None


def _build():
    import concourse.bacc as bacc
    import concourse.bass as bass
    import concourse.mybir as mybir
    import concourse.tile as tile
    from concourse.masks import make_identity

    f32 = mybir.dt.float32
    bf16 = mybir.dt.bfloat16
    f8 = mybir.dt.float8e4
    AF = mybir.ActivationFunctionType
    OP = mybir.AluOpType
    DRW = mybir.MatmulPerfMode.DoubleRow
    ts = bass.ts

    nc = bacc.Bacc()
    x_d = nc.declare_dram_parameter("x", [N, DIM], f32, isOutput=False)
    pos_d = nc.declare_dram_parameter("pos", [N, DIM], f32, isOutput=False)
    maskf_d = nc.declare_dram_parameter("maskf", [N], f32, isOutput=False)
    wqk_d = nc.declare_dram_parameter("W_qk", [2 * DIM, DIM], f32, isOutput=False)
    wv_d = nc.declare_dram_parameter("W_v", [DIM, DIM], f32, isOutput=False)
    wout_d = nc.declare_dram_parameter("W_out", [DIM, DIM], f32, isOutput=False)
    b_d = nc.declare_dram_parameter("b_out", [DIM], f32, isOutput=False)
    out_d = nc.declare_dram_parameter("out", [N, DIM], f32, isOutput=True)

    with ExitStack() as ctx:
        tc = ctx.enter_context(tile.TileContext(nc))
        sing = ctx.enter_context(tc.tile_pool(name="sing", bufs=1))
        expp = ctx.enter_context(tc.tile_pool(name="expp", bufs=4))
        oasp = ctx.enter_context(tc.tile_pool(name="oasp", bufs=2))
        ps_big = ctx.enter_context(tc.tile_pool(name="ps_big", bufs=2, space="PSUM"))

        def mm(out_ap, lhsT, rhs, start, stop):
            """matmul with the free dim chunked to MMN columns (rhs 2D [K, n])."""
            n = rhs.shape[-1]
            if n <= MMN:
                nc.tensor.matmul(out_ap, lhsT, rhs, start=start, stop=stop)
            else:
                for c0 in range(0, n, MMN):
                    nc.tensor.matmul(out_ap[:, c0:c0 + MMN], lhsT,
                                     rhs[:, c0:c0 + MMN], start=start, stop=stop)

        # ---------- constants / small prep ----------
        mstage = sing.tile([P, NT], f32, tag="mstage")
        nc.sync.dma_start(out=mstage, in_=maskf_d.rearrange("(c p) -> p c", c=NT))
        bias_j = sing.tile([P, NT], f32, tag="bias_j")
        nc.scalar.activation(bias_j, mstage, AF.Copy, bias=-MB, scale=MB)
        # preload the exp table set off the critical path
        dume = sing.tile([1, 8], f32, tag="dume")
        nc.scalar.activation(dume, mstage[0:1, :], AF.Exp)
        m_row = sing.tile([1, N], f32, tag="m_row")
        nc.sync.dma_start(out=m_row, in_=maskf_d[:])
        omm_row = sing.tile([1, N], f32, tag="omm_row")
        nc.vector.tensor_scalar(omm_row, m_row, -1.0, 1.0, OP.mult, OP.add)
        ommb = sing.tile([D, N], f32, tag="ommb")
        nc.gpsimd.partition_broadcast(ommb, omm_row)
        m_coll = sing.tile([P, 8], f32, tag="m_coll")
        nc.sync.dma_start(out=m_coll, in_=maskf_d.rearrange("(p c) -> p c", c=8))
        b_row = sing.tile([1, DIM], f32, tag="b_row")
        nc.sync.dma_start(out=b_row, in_=b_d[:])
        b_bcast = sing.tile([P, DIM], f32, tag="b_bcast")
        nc.gpsimd.partition_broadcast(b_bcast, b_row)

        # ---------- persistent data tiles ----------
        V_sb = [sing.tile([P, H * E], bf16, tag=f"V{nt}", name=f"V{nt}")
                for nt in range(NT)]
        qT_all = [sing.tile([P, N], bf16, tag=f"qT{t}", name=f"qT{t}")
                  for t in range(NPAIR)]
        kT_all = [sing.tile([P, N], bf16, tag=f"kT{t}", name=f"kT{t}")
                  for t in range(NPAIR)]
        woutT = sing.tile([P, KT, DIM], bf16, tag="woutT")   # [c, kc, od]
        otfull = [sing.tile([P, N], bf16, tag=f"otf{t}", name=f"otf{t}")
                  for t in range(NPAIR)]
        mean_cols = sing.tile([D, H], f32, tag="mean_cols")

        with (tc.tile_pool(name="p1", bufs=1) as p1,
              tc.tile_pool(name="rowload", bufs=6) as rowload,
              tc.tile_pool(name="woldp", bufs=8) as woldp,
              tc.tile_pool(name="posld", bufs=3) as posld,
              tc.tile_pool(name="wstage", bufs=2) as wstage,
              tc.tile_pool(name="posp", bufs=2) as posp,
              tc.tile_pool(name="ps_tp", bufs=2, space="PSUM") as ps_tp,
              tc.tile_pool(name="ps_sm", bufs=2, space="PSUM") as ps_sm):
            ident_bf = sing.tile([P, P], bf16, tag="ident_bf")
            make_identity(nc, ident_bf)

            # x^T fp8 persists through the DR projections
            xT_f8 = p1.tile([P, KT, N], f8, tag="xT_f8")

            def load_cast(dram_ap, scale=None, pool=None, ldb_bufs=4,
                          eng=None):
                pool = pool or rowload
                r = rowload.tile([P, DIM], f32, tag="ld", name="ld", bufs=2)
                nc.sync.dma_start(out=r, in_=dram_ap)
                bt = pool.tile([P, DIM], bf16, tag="ldb", name="ldb",
                               bufs=ldb_bufs)
                if eng == "gpsimd":
                    nc.gpsimd.tensor_copy(bt, r)
                elif scale is None:
                    nc.scalar.activation(bt, r, AF.Copy)
                else:
                    nc.scalar.activation(bt, r, AF.Copy, scale=scale)
                return bt

            def tp4(src_bf, kc0, dve_dsts, act_dsts=()):
                """Transpose 4 [P,P] blocks (cols kc0*P..) into one PSUM bank,
                then evict once per dst AP ([P, 4, P] view)."""
                tp = ps_tp.tile([P, 4 * P], bf16, tag="tp", name="tp")
                for i in range(4):
                    nc.tensor.matmul(tp[:, ts(i, P)], src_bf[:, ts(kc0 + i, P)],
                                     ident_bf, is_transpose=True,
                                     start=(i == 0), stop=(i == 3),
                                     skip_group_check=True)
                tpv = tp.rearrange("p (k c) -> p k c", c=P)
                for dst in dve_dsts:
                    nc.vector.tensor_copy(dst, tpv)
                for dst in act_dsts:
                    nc.scalar.activation(dst, tpv, AF.Copy)
            const1 = sing.tile([P, H], f32, tag="const1")
            nc.vector.memset(const1, 1.0)
            for nt in range(NT):
                ones_ap = V_sb[nt].rearrange("p (h e) -> p h e", e=E)[:, :, D:E]
                nc.vector.tensor_copy(ones_ap.squeeze(), const1)

            with tc.tile_pool(name="vps", bufs=1) as vps:
                # x^T bf16 and W_v^T live only until the V projection
                xT_bf = vps.tile([P, KT, N], bf16, tag="xT_bf")   # [c, kc, i]
                wvT = vps.tile([P, KT, DIM], bf16, tag="wvT")     # [c, kc, vd]
                for nt in range(NT):
                    xb = load_cast(x_d[ts(nt, P), :])
                    for g in range(2):
                        tp4(xb, 4 * g,
                            [xT_bf[:, 4 * g:4 * g + 4, ts(nt, P)]],
                            [xT_f8[:, 4 * g:4 * g + 4, ts(nt, P)]])
                    wb = load_cast(wv_d[ts(nt, P), :])
                    for g in range(2):
                        tp4(wb, 4 * g, [wvT[:, 4 * g:4 * g + 4, ts(nt, P)]])

                # ---------- V = x @ W_v.T (stored [V_h | 1] per head) ----------
                for nt in range(NT):
                    acc = ps_big.tile([P, N], f32, tag="st", name="vacc")
                    for kc in range(KT):
                        mm(acc, xT_bf[:, kc, ts(nt, P)], wvT[:, kc, :],
                           start=(kc == 0), stop=(kc == KT - 1))
                    dst = V_sb[nt].rearrange("p (h e) -> p h e", e=E)[:, :, 0:D]
                    nc.vector.tensor_copy(dst,
                                          acc.rearrange("p (h d) -> p h d", d=D))

            # ---------- per-pair load prefetch helpers ----------
            def pair_loads(t):
                pr = posld.tile([P, N], f32, tag="posr", name="posr", bufs=2)
                nc.sync.dma_start(
                    out=pr,
                    in_=pos_d.rearrange("(nt p) c -> p nt c", nt=NT)[:, :, ts(t, P)])
                pb = posld.tile([P, N], bf16, tag="posb", name="posb", bufs=2)
                nc.scalar.activation(pb, pr, AF.Copy, scale=WS)
                wq = load_cast(wqk_d[t * P:(t + 1) * P, :], scale=WS)
                wk = load_cast(wqk_d[DIM + t * P:DIM + (t + 1) * P, :], scale=WS)
                return pb, wq, wk

            loads = {0: pair_loads(0), 1: pair_loads(1)}

            # ---------- mean over sequence of V_aug ----------
            ones_col = sing.tile([P, 1], bf16, tag="ones_col")
            constN = sing.tile([P, 1], f32, tag="constN")
            nc.vector.memset(constN, 1.0 / N)
            nc.vector.tensor_copy(ones_col, constN)
            mean_sb = sing.tile([1, H * E], f32, tag="mean_sb")
            for c0, cs in ((0, 512), (512, 512), (1024, H * E - 1024)):
                mp = ps_sm.tile([P, 512], f32, tag="mp", name="mp")
                for nt in range(NT):
                    nc.tensor.matmul(mp[0:1, 0:cs], ones_col, V_sb[nt][:, c0:c0 + cs],
                                     start=(nt == 0), stop=(nt == NT - 1))
                nc.vector.tensor_copy(mean_sb[:, c0:c0 + cs], mp[0:1, 0:cs])
            for h in range(H):
                nc.sync.dma_start(out=mean_cols[:, h:h + 1],
                                  in_=mean_sb[0:1, h * E:h * E + D])

            # ---------- q^T / k^T: fp8 DoubleRow projection + pos add ----------
            wout_b = []
            for t in range(NPAIR):
                if t + 2 < NPAIR:
                    loads[t + 2] = pair_loads(t + 2)
                if t < 4:   # W_out rows: 2 per pair, cast on idle gpsimd
                    for rt in (2 * t, 2 * t + 1):
                        wout_b.append(load_cast(wout_d[ts(rt, P), :], pool=woldp,
                                                ldb_bufs=8, eng="gpsimd"))
                pb, wq, wk = loads.pop(t)
                posT = posp.tile([P, N], bf16, tag="posT", name="posT")
                for g in range(2):
                    tp4(pb, 4 * g,
                        [posT.rearrange("p (nt c) -> p nt c", c=P)[:, 4 * g:4 * g + 4, :]])
                for wqb, wt in ((wq, qT_all[t]), (wk, kT_all[t])):
                    w8 = wstage.tile([P, KT, P], f8, tag="w8", name="w8")
                    tp4(wqb, 0, [w8[:, 0:4, :]])
                    tp4(wqb, 4, [], [w8[:, 4:8, :]])
                    acc = ps_big.tile([P, N], f32, tag="st", name="qacc")
                    for c in range(KT // 2):
                        lhs = w8[:, 2 * c:2 * c + 2, :]
                        for ch in range(2):
                            nc.tensor.matmul(
                                acc[:, ts(ch, MMN)], lhs,
                                xT_f8[:, 2 * c:2 * c + 2, ts(ch, MMN)],
                                start=(c == 0), stop=(c == KT // 2 - 1),
                                perf_mode=DRW)
                    nc.vector.tensor_add(wt, acc, posT)
                if 2 <= t < 6:   # W_out^T transposes, 2 rows per pair
                    for rt in (2 * (t - 2), 2 * (t - 2) + 1):
                        for g in range(2):
                            tp4(wout_b[rt], 4 * g,
                                [woutT[:, 4 * g:4 * g + 4, ts(rt, P)]])

            if DBG == "qT":
                dbg1 = sing.tile([P, N], f32, tag="dbg1")
                nc.vector.tensor_copy(dbg1, qT_all[0])
                nc.sync.dma_start(out=out_d[0:P, :], in_=dbg1)
                nc.vector.tensor_copy(dbg1, kT_all[0])
                nc.sync.dma_start(out=out_d[P:2 * P, :], in_=dbg1)
            if DBG == "V":
                dbg1 = sing.tile([P, N], f32, tag="dbg1")
                nc.vector.tensor_copy(dbg1, V_sb[0][:, 0:N])
                nc.sync.dma_start(out=out_d[0:P, :], in_=dbg1)

        # ---------- attention ----------
        EXPS = SCALE / (WS * WS)

        with (tc.tile_pool(name="normp", bufs=2) as normp,
              tc.tile_pool(name="ps_oa", bufs=2, space="PSUM") as ps_oa):

            def norm_head(t, hs, oa):
                """Normalize head (2t+hs) from PSUM oa [65, N] into otfull[t]."""
                h = 2 * t + hs
                oaS = oasp.tile([E, N], bf16, tag="oaS", name="oaS")
                nc.vector.tensor_copy(oaS, oa)  # frees the PSUM banks
                s_coll = normp.tile([P, 8], bf16, tag="s_coll")
                nc.sync.dma_start(out=s_coll, in_=oaS[D:D + 1, :])
                r_coll = normp.tile([P, 8], f32, tag="r_coll")
                nc.vector.reciprocal(r_coll, s_coll)
                nc.vector.tensor_mul(r_coll, r_coll, m_coll)
                rm_row = normp.tile([1, N], f32, tag="rm_row")
                nc.sync.dma_start(
                    out=rm_row.rearrange("o (p c) -> o p c", p=P, c=8),
                    in_=r_coll)
                rmb = normp.tile([D, N], f32, tag="rmb")
                nc.gpsimd.partition_broadcast(rmb, rm_row)
                t1 = normp.tile([D, N], f32, tag="t1")
                nc.vector.tensor_mul(t1, oaS[0:D, :], rmb)
                if hs == 0:
                    nc.vector.scalar_tensor_tensor(
                        otfull[t][0:D, :], ommb, mean_cols[:, h:h + 1], t1,
                        OP.mult, OP.add)
                else:
                    hscr = normp.tile([D, N], bf16, tag="hscr")
                    nc.vector.scalar_tensor_tensor(
                        hscr, ommb, mean_cols[:, h:h + 1], t1, OP.mult, OP.add)
                    nc.sync.dma_start(out=otfull[t][D:2 * D, :], in_=hscr)

            def pv(t, prev, stop):
                pj, pexA, pexB = prev
                hA, hB = 2 * t, 2 * t + 1
                mm(oa_ab[0], V_sb[pj][:, hA * E:(hA + 1) * E], pexA,
                   start=(pj == 0), stop=stop)
                mm(oa_ab[1], V_sb[pj][:, hB * E:(hB + 1) * E], pexB,
                   start=(pj == 0), stop=stop)

            def oproj_mm(acc, nt, kcs, start):
                for kc in kcs:
                    mm(acc, otfull[kc][:, ts(nt, P)], woutT[:, kc, :],
                       start=(start and kc == kcs[0]), stop=(kc == KT - 1))

            def oproj_fin(acc, nt):
                ostage = normp.tile([P, N], f32, tag="ostage", name="ostage")
                nc.vector.tensor_add(ostage, acc, b_bcast)
                nc.sync.dma_start(out=out_d[ts(nt, P), :], in_=ostage)

            oacc4 = []
            for t in range(NPAIR):
                qT, kT = qT_all[t], kT_all[t]
                oa_ab = (ps_oa.tile([E, N], f32, tag="oa", name="oaA"),
                         ps_oa.tile([E, N], f32, tag="oa", name="oaB"))
                prev = None
                for jt in range(NT):
                    stA = ps_big.tile([P, N], f32, tag="st", name="stA")
                    mm(stA, kT[0:D, ts(jt, P)], qT[0:D, :], start=True, stop=True)
                    stB = ps_big.tile([P, N], f32, tag="st", name="stB")
                    mm(stB, kT[D:P, ts(jt, P)], qT[D:P, :], start=True, stop=True)
                    if prev is not None:
                        pv(t, prev, stop=False)
                    exA = expp.tile([P, N], bf16, tag="ex", name="exA")
                    nc.scalar.activation(exA, stA, AF.Exp,
                                         bias=bias_j[:, jt:jt + 1], scale=EXPS)
                    exB = expp.tile([P, N], bf16, tag="ex", name="exB")
                    nc.scalar.activation(exB, stB, AF.Exp,
                                         bias=bias_j[:, jt:jt + 1], scale=EXPS)
                    if DBG == "ex" and t == 0 and jt == 0:
                        dbg1 = sing.tile([P, N], f32, tag="dbg1")
                        nc.vector.tensor_copy(dbg1, stA)
                        nc.sync.dma_start(out=out_d[0:P, :], in_=dbg1)
                        dbg2 = sing.tile([P, N], f32, tag="dbg2")
                        nc.vector.tensor_copy(dbg2, exA)
                        nc.sync.dma_start(out=out_d[P:2 * P, :], in_=dbg2)
                    prev = (jt, exA, exB)
                pv(t, prev, stop=True)
                last = t == NPAIR - 1 and not DBG
                if last:
                    # keep PE busy during the last pair's normalization:
                    # accumulate pairs 0..6 of the first four out row-tiles
                    for nt in range(2):
                        acc = ps_big.tile([P, N], f32, tag="st", name="oacc")
                        oproj_mm(acc, nt, list(range(KT - 1)), start=True)
                        oacc4.append(acc)
                # for the last pair normalize head B first: its otfull write
                # goes through a DMA and gates the final out-proj kc
                for hs in ((1, 0) if last else (0, 1)):
                    norm_head(t, hs, oa_ab[hs])
                if last:
                    # reuse the oa slots (freed by the oaS evictions just above)
                    for nt in range(2, 4):
                        acc = ps_oa.tile([P, N], f32, tag="oa", name="oacc2")
                        oproj_mm(acc, nt, list(range(KT - 1)), start=True)
                        oacc4.append(acc)

            if DBG == "ot":
                dbg3 = sing.tile([P, N], f32, tag="dbg3")
                for kc in range(KT):
                    nc.vector.tensor_copy(dbg3, otfull[kc])
                    nc.sync.dma_start(out=out_d[ts(kc, P), :], in_=dbg3)

            # ---------- out projection (remaining) ----------
            if not DBG:
                for nt in range(4):
                    oproj_mm(oacc4[nt], nt, [KT - 1], start=False)
                    oproj_fin(oacc4[nt], nt)
                for nt in range(4, NT):
                    acc = ps_big.tile([P, N], f32, tag="st", name="oacc")
                    oproj_mm(acc, nt, list(range(KT)), start=True)
                    oproj_fin(acc, nt)

    nc.finalize()
    return nc


def kernel(x, mask, pos, W_qk, W_v, W_out, b_out):
    global _NC
    from concourse.bass_utils import run_bass_kernel_spmd

    if _NC is None:
        _NC = _build()

    x = np.ascontiguousarray(x, dtype=np.float32)
    pos = np.ascontiguousarray(pos, dtype=np.float32)
    maskf = np.concatenate(
        [np.ones((B, 1), np.float32), np.asarray(mask).astype(np.float32)], axis=1)
    W_qk = np.ascontiguousarray(W_qk, dtype=np.float32)
    W_v = np.ascontiguousarray(W_v, dtype=np.float32)
    W_out = np.ascontiguousarray(W_out, dtype=np.float32)
    b_out = np.ascontiguousarray(b_out, dtype=np.float32)

    in_maps = [
        {"x": x[b], "pos": pos[b], "maskf": maskf[b], "W_qk": W_qk,
         "W_v": W_v, "W_out": W_out, "b_out": b_out}
        for b in range(B)
    ]
    res = run_bass_kernel_spmd(_NC, in_maps, core_ids=list(range(B)))
    return np.stack([res.results[b]["out"] for b in range(B)]).astype(np.float32)


# revision 30
# speedup vs baseline: 1.2305x; 1.0340x over previous
"""Multi-head attention kernel for Trainium2, batch-parallel across 8 NeuronCores.

Reference (per batch element b, one core each):
  qk = x @ W_qk.T ; q,k = split(qk) ; v = x @ W_v.T
  q,k,v -> [h, n, d] ; q += pos_h ; k += pos_h
  S = q @ k.T * DIM**-0.5 ; mask = outer(m, m) ; masked -> -inf
  P = softmax(S) ; O = P @ v ; out = merge_heads(O) @ W_out.T + b_out

Device strategy (per core), v4:
  - Phase 1 (prep): x^T kept in bf16 (for V=x@W_v.T) and fp8e4 (for the q/k
    projection, fp8 DoubleRow with W_qk prescaled by 32 -- 256-deep
    contraction per pass).  Per-pair pos/W_qk loads are prefetched two pairs
    ahead so the single Sync DMA queue never head-of-line-blocks the PE;
    transpose evictions are split between ACT and DVE.
  - Phase 2 (attention): per head pair, software-pipelined over 8 j-tiles:
    scores on PE (kT-A rows 0-63 / kT-B rows 64-127 alternate so LDWEIGHTS
    hides), exp on ACT (mask bias folded in, scale/1024), PV accumulating
    [65,1024] in PSUM (ones col = row sums).  Normalization is DMA-free:
    reciprocal+mask on the [1,N] sums row, gpsimd broadcast, one mul + one
    scalar_tensor_tensor per head.
  - Phase 3: out projection; four row-tiles accumulate pairs 0-6 early
    (overlapping the last pair's normalization) so PE never idles/cools.
"""
import os
import sys

sys.path.insert(0, "/opt/trn_rl_repo")

import numpy as np
from contextlib import ExitStack

DBG = os.environ.get("KDBG", "")

B, N, DIM, H = 8, 1024, 1024, 16
D = DIM // H          # 64
E = D + 1             # V_aug block (64 cols of V + ones column)
P = 128
NT = N // P           # 8 n-tiles
KT = DIM // P         # 8 k-tiles
NPAIR = H // 2        # 8 head pairs
SCALE = DIM ** (-0.5)
MB = 30.0             # mask bias magnitude: bias_j = 30*m - 30 in {0, -30}
MMN = 512             # moving free-dim per matmul (single-bank PSUM writes)
WS = 32.0             # fp8 prescale on W_qk / pos (q' = 32 q); exp scale /1024

_NC = None


def _build():
    import concourse.bacc as bacc
    import concourse.bass as bass
    import concourse.mybir as mybir
    import concourse.tile as tile
    from concourse.masks import make_identity

    f32 = mybir.dt.float32
    bf16 = mybir.dt.bfloat16
    f8 = mybir.dt.float8e4
    AF = mybir.ActivationFunctionType
    OP = mybir.AluOpType
    DRW = mybir.MatmulPerfMode.DoubleRow
    ts = bass.ts

    nc = bacc.Bacc()
    x_d = nc.declare_dram_parameter("x", [N, DIM], f32, isOutput=False)
    pos_d = nc.declare_dram_parameter("pos", [N, DIM], f32, isOutput=False)
    maskf_d = nc.declare_dram_parameter("maskf", [N], f32, isOutput=False)
    wqk_d = nc.declare_dram_parameter("W_qk", [2 * DIM, DIM], f32, isOutput=False)
    wv_d = nc.declare_dram_parameter("W_v", [DIM, DIM], f32, isOutput=False)
    wout_d = nc.declare_dram_parameter("W_out", [DIM, DIM], f32, isOutput=False)
    b_d = nc.declare_dram_parameter("b_out", [DIM], f32, isOutput=False)
    out_d = nc.declare_dram_parameter("out", [N, DIM], f32, isOutput=True)

    with ExitStack() as ctx:
        tc = ctx.enter_context(tile.TileContext(nc))
        sing = ctx.enter_context(tc.tile_pool(name="sing", bufs=1))
        expp = ctx.enter_context(tc.tile_pool(name="expp", bufs=4))
        oasp = ctx.enter_context(tc.tile_pool(name="oasp", bufs=2))
        ps_big = ctx.enter_context(tc.tile_pool(name="ps_big", bufs=2, space="PSUM"))

        def mm(out_ap, lhsT, rhs, start, stop):
            """matmul with the free dim chunked to MMN columns (rhs 2D [K, n])."""
            n = rhs.shape[-1]
            if n <= MMN:
                nc.tensor.matmul(out_ap, lhsT, rhs, start=start, stop=stop)
            else:
                for c0 in range(0, n, MMN):
                    nc.tensor.matmul(out_ap[:, c0:c0 + MMN], lhsT,
                                     rhs[:, c0:c0 + MMN], start=start, stop=stop)

        # ---------- constants / small prep ----------
        mstage = sing.tile([P, NT], f32, tag="mstage")
        nc.sync.dma_start(out=mstage, in_=maskf_d.rearrange("(c p) -> p c", c=NT))
        bias_j = sing.tile([P, NT], f32, tag="bias_j")
        nc.scalar.activation(bias_j, mstage, AF.Copy, bias=-MB, scale=MB)
        # preload the exp table set off the critical path
        dume = sing.tile([1, 8], f32, tag="dume")
        nc.scalar.activation(dume, mstage[0:1, :], AF.Exp)
        m_row = sing.tile([1, N], f32, tag="m_row")
        nc.sync.dma_start(out=m_row, in_=maskf_d[:])
        omm_row = sing.tile([1, N], f32, tag="omm_row")
        nc.vector.tensor_scalar(omm_row, m_row, -1.0, 1.0, OP.mult, OP.add)
        ommb = sing.tile([D, N], f32, tag="ommb")
        nc.gpsimd.partition_broadcast(ommb, omm_row)
        m_coll = sing.tile([P, 8], f32, tag="m_coll")
        nc.sync.dma_start(out=m_coll, in_=maskf_d.rearrange("(p c) -> p c", c=8))
        b_row = sing.tile([1, DIM], f32, tag="b_row")
        nc.sync.dma_start(out=b_row, in_=b_d[:])
        b_bcast = sing.tile([P, DIM], f32, tag="b_bcast")
        nc.gpsimd.partition_broadcast(b_bcast, b_row)

        # ---------- persistent data tiles ----------
        V_sb = [sing.tile([P, H * E], bf16, tag=f"V{nt}", name=f"V{nt}")
                for nt in range(NT)]
        qT_all = [sing.tile([P, N], bf16, tag=f"qT{t}", name=f"qT{t}")
                  for t in range(NPAIR)]
        kT_all = [sing.tile([P, N], bf16, tag=f"kT{t}", name=f"kT{t}")
                  for t in range(NPAIR)]
        woutT = sing.tile([P, KT, DIM], bf16, tag="woutT")   # [c, kc, od]
        otfull = [sing.tile([P, N], bf16, tag=f"otf{t}", name=f"otf{t}")
                  for t in range(NPAIR)]
        mean_cols = sing.tile([D, H], f32, tag="mean_cols")

        with (tc.tile_pool(name="p1", bufs=1) as p1,
              tc.tile_pool(name="rowload", bufs=6) as rowload,
              tc.tile_pool(name="woldp", bufs=6) as woldp,
              tc.tile_pool(name="posld", bufs=3) as posld,
              tc.tile_pool(name="wstage", bufs=2) as wstage,
              tc.tile_pool(name="posp", bufs=2) as posp,
              tc.tile_pool(name="ps_tp", bufs=2, space="PSUM") as ps_tp,
              tc.tile_pool(name="ps_sm", bufs=2, space="PSUM") as ps_sm):
            ident_bf = sing.tile([P, P], bf16, tag="ident_bf")
            make_identity(nc, ident_bf)

            # x^T fp8 persists through the DR projections
            xT_f8 = p1.tile([P, KT, N], f8, tag="xT_f8")

            def load_cast(dram_ap, scale=None, pool=None, ldb_bufs=4,
                          eng=None):
                pool = pool or rowload
                r = rowload.tile([P, DIM], f32, tag="ld", name="ld", bufs=2)
                nc.sync.dma_start(out=r, in_=dram_ap)
                bt = pool.tile([P, DIM], bf16, tag="ldb", name="ldb",
                               bufs=ldb_bufs)
                if eng == "gpsimd":
                    nc.gpsimd.tensor_copy(bt, r)
                elif scale is None:
                    nc.scalar.activation(bt, r, AF.Copy)
                else:
                    nc.scalar.activation(bt, r, AF.Copy, scale=scale)
                return bt

            def tp4(src_bf, kc0, dve_dsts, act_dsts=()):
                """Transpose 4 [P,P] blocks (cols kc0*P..) into one PSUM bank,
                then evict once per dst AP ([P, 4, P] view)."""
                tp = ps_tp.tile([P, 4 * P], bf16, tag="tp", name="tp")
                for i in range(4):
                    nc.tensor.matmul(tp[:, ts(i, P)], src_bf[:, ts(kc0 + i, P)],
                                     ident_bf, is_transpose=True,
                                     start=(i == 0), stop=(i == 3),
                                     skip_group_check=True)
                tpv = tp.rearrange("p (k c) -> p k c", c=P)
                for dst in dve_dsts:
                    nc.vector.tensor_copy(dst, tpv)
                for dst in act_dsts:
                    nc.scalar.activation(dst, tpv, AF.Copy)
            const1 = sing.tile([P, H], f32, tag="const1")
            nc.vector.memset(const1, 1.0)
            for nt in range(NT):
                ones_ap = V_sb[nt].rearrange("p (h e) -> p h e", e=E)[:, :, D:E]
                nc.vector.tensor_copy(ones_ap.squeeze(), const1)

            with tc.tile_pool(name="vps", bufs=1) as vps:
                # x^T bf16 and W_v^T live only until the V projection
                xT_bf = vps.tile([P, KT, N], bf16, tag="xT_bf")   # [c, kc, i]
                wvT = vps.tile([P, KT, DIM], bf16, tag="wvT")     # [c, kc, vd]
                # W_v first, then x tiles with the V projection trailing each
                for rt in range(NT):
                    wb = load_cast(wv_d[ts(rt, P), :])
                    for g in range(2):
                        tp4(wb, 4 * g, [wvT[:, 4 * g:4 * g + 4, ts(rt, P)]])
                for nt in range(NT):
                    xb = load_cast(x_d[ts(nt, P), :])
                    for g in range(2):
                        tp4(xb, 4 * g,
                            [xT_bf[:, 4 * g:4 * g + 4, ts(nt, P)]],
                            [xT_f8[:, 4 * g:4 * g + 4, ts(nt, P)]])
                    acc = ps_big.tile([P, N], f32, tag="st", name="vacc")
                    for kc in range(KT):
                        mm(acc, xT_bf[:, kc, ts(nt, P)], wvT[:, kc, :],
                           start=(kc == 0), stop=(kc == KT - 1))
                    dst = V_sb[nt].rearrange("p (h e) -> p h e", e=E)[:, :, 0:D]
                    nc.vector.tensor_copy(dst,
                                          acc.rearrange("p (h d) -> p h d", d=D))

            # ---------- per-pair load prefetch helpers ----------
            def pair_loads(t):
                pr = posld.tile([P, N], f32, tag="posr", name="posr", bufs=2)
                nc.sync.dma_start(
                    out=pr,
                    in_=pos_d.rearrange("(nt p) c -> p nt c", nt=NT)[:, :, ts(t, P)])
                pb = posld.tile([P, N], bf16, tag="posb", name="posb", bufs=3)
                nc.scalar.activation(pb, pr, AF.Copy, scale=WS)
                wq = load_cast(wqk_d[t * P:(t + 1) * P, :], scale=WS)
                wk = load_cast(wqk_d[DIM + t * P:DIM + (t + 1) * P, :], scale=WS)
                return pb, wq, wk

            loads = {0: pair_loads(0), 1: pair_loads(1)}

            # ---------- mean over sequence of V_aug ----------
            ones_col = sing.tile([P, 1], bf16, tag="ones_col")
            constN = sing.tile([P, 1], f32, tag="constN")
            nc.vector.memset(constN, 1.0 / N)
            nc.vector.tensor_copy(ones_col, constN)
            mean_sb = sing.tile([1, H * E], f32, tag="mean_sb")
            for c0, cs in ((0, 512), (512, 512), (1024, H * E - 1024)):
                mp = ps_sm.tile([P, 512], f32, tag="mp", name="mp")
                for nt in range(NT):
                    nc.tensor.matmul(mp[0:1, 0:cs], ones_col, V_sb[nt][:, c0:c0 + cs],
                                     start=(nt == 0), stop=(nt == NT - 1))
                nc.vector.tensor_copy(mean_sb[:, c0:c0 + cs], mp[0:1, 0:cs])


            # ---------- q^T / k^T: fp8 DoubleRow projection + pos add ----------
            wout_b = []
            for t in range(NPAIR):
                if t + 2 < NPAIR:
                    loads[t + 2] = pair_loads(t + 2)
                if t < 4:   # W_out rows: 2 per pair, cast on idle gpsimd
                    for rt in (2 * t, 2 * t + 1):
                        wout_b.append(load_cast(wout_d[ts(rt, P), :], pool=woldp,
                                                ldb_bufs=6, eng="gpsimd"))
                pb, wq, wk = loads.pop(t)
                posT = posp.tile([P, N], bf16, tag="posT", name="posT")
                for g in range(2):
                    tp4(pb, 4 * g,
                        [posT.rearrange("p (nt c) -> p nt c", c=P)[:, 4 * g:4 * g + 4, :]])
                for wqb, wt in ((wq, qT_all[t]), (wk, kT_all[t])):
                    w8 = wstage.tile([P, KT, P], f8, tag="w8", name="w8")
                    tp4(wqb, 0, [w8[:, 0:4, :]])
                    tp4(wqb, 4, [], [w8[:, 4:8, :]])
                    acc = ps_big.tile([P, N], f32, tag="st", name="qacc")
                    for c in range(KT // 2):
                        lhs = w8[:, 2 * c:2 * c + 2, :]
                        for ch in range(2):
                            nc.tensor.matmul(
                                acc[:, ts(ch, MMN)], lhs,
                                xT_f8[:, 2 * c:2 * c + 2, ts(ch, MMN)],
                                start=(c == 0), stop=(c == KT // 2 - 1),
                                perf_mode=DRW)
                    nc.vector.tensor_add(wt, acc, posT)
                if 2 <= t < 6:   # W_out^T transposes, 2 rows per pair
                    for rt in (2 * (t - 2), 2 * (t - 2) + 1):
                        for g in range(2):
                            tp4(wout_b[rt], 4 * g,
                                [woutT[:, 4 * g:4 * g + 4, ts(rt, P)]])

            # mean gather: emitted last so the tiny DMAs never head-of-line
            # block the pair loads on the sync queue (first use is pair-0 norm)
            for h in range(H):
                nc.sync.dma_start(out=mean_cols[:, h:h + 1],
                                  in_=mean_sb[0:1, h * E:h * E + D])

            if DBG == "qT":
                dbg1 = sing.tile([P, N], f32, tag="dbg1")
                nc.vector.tensor_copy(dbg1, qT_all[0])
                nc.sync.dma_start(out=out_d[0:P, :], in_=dbg1)
                nc.vector.tensor_copy(dbg1, kT_all[0])
                nc.sync.dma_start(out=out_d[P:2 * P, :], in_=dbg1)
            if DBG == "V":
                dbg1 = sing.tile([P, N], f32, tag="dbg1")
                nc.vector.tensor_copy(dbg1, V_sb[0][:, 0:N])
                nc.sync.dma_start(out=out_d[0:P, :], in_=dbg1)

        # ---------- attention ----------
        EXPS = SCALE / (WS * WS)

        with (tc.tile_pool(name="normp", bufs=2) as normp,
              tc.tile_pool(name="ps_oa", bufs=2, space="PSUM") as ps_oa):

            def norm_head(t, hs, oa):
                """Normalize head (2t+hs) from PSUM oa [65, N] into otfull[t]."""
                h = 2 * t + hs
                oaS = oasp.tile([E, N], bf16, tag="oaS", name="oaS")
                nc.vector.tensor_copy(oaS, oa)  # frees the PSUM banks
                s_coll = normp.tile([P, 8], bf16, tag="s_coll")
                nc.sync.dma_start(out=s_coll, in_=oaS[D:D + 1, :])
                r_coll = normp.tile([P, 8], f32, tag="r_coll")
                nc.vector.reciprocal(r_coll, s_coll)
                nc.vector.tensor_mul(r_coll, r_coll, m_coll)
                rm_row = normp.tile([1, N], f32, tag="rm_row")
                nc.sync.dma_start(
                    out=rm_row.rearrange("o (p c) -> o p c", p=P, c=8),
                    in_=r_coll)
                rmb = normp.tile([D, N], f32, tag="rmb")
                nc.gpsimd.partition_broadcast(rmb, rm_row)
                t1 = normp.tile([D, N], f32, tag="t1")
                nc.vector.tensor_mul(t1, oaS[0:D, :], rmb)
                if hs == 0:
                    nc.vector.scalar_tensor_tensor(
                        otfull[t][0:D, :], ommb, mean_cols[:, h:h + 1], t1,
                        OP.mult, OP.add)
                else:
                    hscr = normp.tile([D, N], bf16, tag="hscr")
                    nc.vector.scalar_tensor_tensor(
                        hscr, ommb, mean_cols[:, h:h + 1], t1, OP.mult, OP.add)
                    nc.sync.dma_start(out=otfull[t][D:2 * D, :], in_=hscr)

            def pv(t, prev, stop):
                pj, pexA, pexB = prev
                hA, hB = 2 * t, 2 * t + 1
                mm(oa_ab[0], V_sb[pj][:, hA * E:(hA + 1) * E], pexA,
                   start=(pj == 0), stop=stop)
                mm(oa_ab[1], V_sb[pj][:, hB * E:(hB + 1) * E], pexB,
                   start=(pj == 0), stop=stop)

            def oproj_mm(acc, nt, kcs, start):
                for kc in kcs:
                    mm(acc, otfull[kc][:, ts(nt, P)], woutT[:, kc, :],
                       start=(start and kc == kcs[0]), stop=(kc == KT - 1))

            def oproj_fin(acc, nt):
                ostage = normp.tile([P, N], f32, tag="ostage", name="ostage")
                nc.vector.tensor_add(ostage, acc, b_bcast)
                nc.sync.dma_start(out=out_d[ts(nt, P), :], in_=ostage)

            oacc4 = []
            for t in range(NPAIR):
                qT, kT = qT_all[t], kT_all[t]
                oa_ab = (ps_oa.tile([E, N], f32, tag="oa", name="oaA"),
                         ps_oa.tile([E, N], f32, tag="oa", name="oaB"))
                prev = None
                for jt in range(NT):
                    stA = ps_big.tile([P, N], f32, tag="st", name="stA")
                    mm(stA, kT[0:D, ts(jt, P)], qT[0:D, :], start=True, stop=True)
                    stB = ps_big.tile([P, N], f32, tag="st", name="stB")
                    mm(stB, kT[D:P, ts(jt, P)], qT[D:P, :], start=True, stop=True)
                    if prev is not None:
                        pv(t, prev, stop=False)
                    exA = expp.tile([P, N], bf16, tag="ex", name="exA")
                    nc.scalar.activation(exA, stA, AF.Exp,
                                         bias=bias_j[:, jt:jt + 1], scale=EXPS)
                    exB = expp.tile([P, N], bf16, tag="ex", name="exB")
                    nc.scalar.activation(exB, stB, AF.Exp,
                                         bias=bias_j[:, jt:jt + 1], scale=EXPS)
                    if DBG == "ex" and t == 0 and jt == 0:
                        dbg1 = sing.tile([P, N], f32, tag="dbg1")
                        nc.vector.tensor_copy(dbg1, stA)
                        nc.sync.dma_start(out=out_d[0:P, :], in_=dbg1)
                        dbg2 = sing.tile([P, N], f32, tag="dbg2")
                        nc.vector.tensor_copy(dbg2, exA)
                        nc.sync.dma_start(out=out_d[P:2 * P, :], in_=dbg2)
                    prev = (jt, exA, exB)
                pv(t, prev, stop=True)
                last = t == NPAIR - 1 and not DBG
                if last:
                    # keep PE busy during the last pair's normalization:
                    # accumulate pairs 0..6 of the first four out row-tiles
                    for nt in range(2):
                        acc = ps_big.tile([P, N], f32, tag="st", name="oacc")
                        oproj_mm(acc, nt, list(range(KT - 1)), start=True)
                        oacc4.append(acc)
                # for the last pair normalize head B first: its otfull write
                # goes through a DMA and gates the final out-proj kc
                for hs in ((1, 0) if last else (0, 1)):
                    norm_head(t, hs, oa_ab[hs])
                if last:
                    # reuse the oa slots (freed by the oaS evictions just above)
                    for nt in range(2, 4):
                        acc = ps_oa.tile([P, N], f32, tag="oa", name="oacc2")
                        oproj_mm(acc, nt, list(range(KT - 1)), start=True)
                        oacc4.append(acc)

            if DBG == "ot":
                dbg3 = sing.tile([P, N], f32, tag="dbg3")
                for kc in range(KT):
                    nc.vector.tensor_copy(dbg3, otfull[kc])
                    nc.sync.dma_start(out=out_d[ts(kc, P), :], in_=dbg3)

            # ---------- out projection (remaining) ----------
            if not DBG:
                for nt in range(4):
                    oproj_mm(oacc4[nt], nt, [KT - 1], start=False)
                    oproj_fin(oacc4[nt], nt)
                for nt in range(4, NT):
                    acc = ps_big.tile([P, N], f32, tag="st", name="oacc")
                    oproj_mm(acc, nt, list(range(KT)), start=True)
                    oproj_fin(acc, nt)

    nc.finalize()
    return nc


def kernel(x, mask, pos, W_qk, W_v, W_out, b_out):
    global _NC
    from concourse.bass_utils import run_bass_kernel_spmd

    if _NC is None:
        _NC = _build()

    x = np.ascontiguousarray(x, dtype=np.float32)
    pos = np.ascontiguousarray(pos, dtype=np.float32)
    maskf = np.concatenate(
        [np.ones((B, 1), np.float32), np.asarray(mask).astype(np.float32)], axis=1)
    W_qk = np.ascontiguousarray(W_qk, dtype=np.float32)
    W_v = np.ascontiguousarray(W_v, dtype=np.float32)
    W_out = np.ascontiguousarray(W_out, dtype=np.float32)
    b_out = np.ascontiguousarray(b_out, dtype=np.float32)

    in_maps = [
        {"x": x[b], "pos": pos[b], "maskf": maskf[b], "W_qk": W_qk,
         "W_v": W_v, "W_out": W_out, "b_out": b_out}
        for b in range(B)
    ]
    res = run_bass_kernel_spmd(_NC, in_maps, core_ids=list(range(B)))
    return np.stack([res.results[b]["out"] for b in range(B)]).astype(np.float32)
